# revision 11
# baseline (speedup 1.0000x reference)
"""Trainium2 Bass kernel for nn_Controller_88862873354711 (NAS RNN controller).

Contract: kernel(**inputs) takes the FULL unsharded inputs (as produced by
setup_inputs()) and returns the full output tuple
(prev_node_ids int32[11], operations int32[12], log_prob f32 scalar).

Strategy
--------
The computation is a strictly sequential chain (24 stacked-LSTM steps with
23 categorical sampling events whose outcomes feed back as embeddings), so it
is replicated on all 8 cores (sharding_hint: not shardable; batch=1).

All RNG is precomputed on host: jax.random.categorical(key, logits) ==
argmax(logits + gumbel(key, shape)), and the key-split sequence is
data-independent, so the 23 gumbel vectors are extracted up front and shipped
to the device as constants.  The device then runs the whole controller as a
deterministic unrolled program in column-vector layout ([100,1] tiles):

  - LSTM cell: 8 PE matmuls accumulate the 4 gates into PSUM [100,4];
    gate nonlinearities are ACT ops with the (b_ih+b_hh) bias fused;
    c/h updates are DVE ops. Only Sigmoid/Tanh are used on-device (one ACT
    table set - no table-swap stalls).
  - Sampling: logits row [1,n] via PE; scaled-tanh via ACT; gumbel add +
    row-max + is_equal give a one-hot row; a K=1 matmul against a ones[1,1]
    transposes it to a column; one more matmul selects the embedding row.
  - Device outputs: tanh'd logit rows and one-hot rows for all 23 events.
    Host computes argmax indices + log-softmax log_prob from those (exp/log
    never run on device).
"""
import sys

import numpy as np

if "/opt/trn_rl_repo" not in sys.path:
    sys.path.insert(0, "/opt/trn_rl_repo")

import concourse.bass as bass  # noqa: E402
import concourse.mybir as mybir  # noqa: E402
from concourse import bacc  # noqa: E402
from concourse.bass_utils import run_bass_kernel_spmd  # noqa: E402
from concourse.tile import TileContext  # noqa: E402

NUM_NODES = 12
NUM_OPS = 8
H = 100
L = 2
TANH_C = 2.5
TEMP = 5.0
N_CORES = 8

F32 = mybir.dt.float32
AF = mybir.ActivationFunctionType
AX = mybir.AxisListType
ALU = mybir.AluOpType

_CACHE = {}


def _build_nc():
    """Build + compile the Bass module (once per process)."""
    nc = bacc.Bacc("TRN2", target_bir_lowering=False, debug=False)

    d_wih = nc.declare_dram_parameter("wih_t", [L, H, 4 * H], F32, isOutput=False)
    d_whh = nc.declare_dram_parameter("whh_t", [L, H, 4 * H], F32, isOutput=False)
    d_bcol = nc.declare_dram_parameter("b_col", [H, 4 * L], F32, isOutput=False)
    d_wap = nc.declare_dram_parameter("wap_t", [H, H], F32, isOutput=False)
    d_bap = nc.declare_dram_parameter("bap_col", [H, 1], F32, isOutput=False)
    d_wac = nc.declare_dram_parameter("wac_t", [H, H], F32, isOutput=False)
    d_bac = nc.declare_dram_parameter("bac_col", [H, 1], F32, isOutput=False)
    d_wpd = nc.declare_dram_parameter("wpd_col", [H, 1], F32, isOutput=False)
    d_bpd5 = nc.declare_dram_parameter("bpd5", [1, 1], F32, isOutput=False)
    d_wod = nc.declare_dram_parameter("wod_t", [H, NUM_OPS], F32, isOutput=False)
    d_bod = nc.declare_dram_parameter("bod_row", [1, NUM_OPS], F32, isOutput=False)
    d_embp = nc.declare_dram_parameter("embp", [NUM_NODES - 1, H], F32, isOutput=False)
    d_embo = nc.declare_dram_parameter("embo", [NUM_OPS, H], F32, isOutput=False)
    d_gprev = nc.declare_dram_parameter("gprev", [1, 16 * (NUM_NODES - 1)], F32, isOutput=False)
    d_gop = nc.declare_dram_parameter("gop", [1, NUM_OPS * NUM_NODES], F32, isOutput=False)

    o_tprev = nc.declare_dram_parameter("t_prev", [1, 16 * (NUM_NODES - 1)], F32, isOutput=True)
    o_ohprev = nc.declare_dram_parameter("oh_prev", [1, 16 * (NUM_NODES - 1)], F32, isOutput=True)
    o_top = nc.declare_dram_parameter("t_op", [1, NUM_OPS * NUM_NODES], F32, isOutput=True)
    o_ohop = nc.declare_dram_parameter("oh_op", [1, NUM_OPS * NUM_NODES], F32, isOutput=True)

    with TileContext(nc) as tc:
        with tc.tile_pool(name="w", bufs=1) as wp, \
             tc.tile_pool(name="st", bufs=3) as st, \
             tc.tile_pool(name="pg", bufs=2, space="PSUM") as pg, \
             tc.tile_pool(name="pv", bufs=2, space="PSUM") as pv:

            def wtile(tag, shape, src):
                t = wp.tile(shape, F32, tag=tag)
                nc.sync.dma_start(out=t[:], in_=src)
                return t

            wih = [wtile(f"wih{l}", [H, 4 * H], d_wih[l]) for l in range(L)]
            whh = [wtile(f"whh{l}", [H, 4 * H], d_whh[l]) for l in range(L)]
            bcol = wtile("bcol", [H, 4 * L], d_bcol[:])
            wap = wtile("wap", [H, H], d_wap[:])
            bap = wtile("bap", [H, 1], d_bap[:])
            wac = wtile("wac", [H, H], d_wac[:])
            bac = wtile("bac", [H, 1], d_bac[:])
            wpd = wtile("wpd", [H, 1], d_wpd[:])
            bpd5 = wtile("bpd5", [1, 1], d_bpd5[:])
            wod = wtile("wod", [H, NUM_OPS], d_wod[:])
            bodr = wtile("bodr", [1, NUM_OPS], d_bod[:])
            embp = wtile("embp", [NUM_NODES - 1, H], d_embp[:])
            embo = wtile("embo", [NUM_OPS, H], d_embo[:])
            gprev = wtile("gprev", [1, 16 * (NUM_NODES - 1)], d_gprev[:])
            gop = wtile("gop", [1, NUM_OPS * NUM_NODES], d_gop[:])

            ones = wp.tile([1, 1], F32, tag="ones")
            nc.vector.memset(ones[:], 1.0)

            tprev_sb = wp.tile([1, 16 * (NUM_NODES - 1)], F32, tag="tprev")
            nc.vector.memset(tprev_sb[:], 0.0)
            ohprev_sb = wp.tile([1, 16 * (NUM_NODES - 1)], F32, tag="ohprev")
            nc.vector.memset(ohprev_sb[:], 0.0)
            top_sb = wp.tile([1, NUM_OPS * NUM_NODES], F32, tag="top")
            ohop_sb = wp.tile([1, NUM_OPS * NUM_NODES], F32, tag="ohop")
            P_sb = wp.tile([H, NUM_NODES], F32, tag="P")

            x = st.tile([H, 1], F32, tag="x")
            nc.vector.memset(x[:], 0.0)
            h, c = [], []
            for l in range(L):
                hl = st.tile([H, 1], F32, tag=f"h{l}")
                nc.vector.memset(hl[:], 0.0)
                h.append(hl)
                cl = st.tile([H, 1], F32, tag=f"c{l}")
                nc.vector.memset(cl[:], 0.0)
                c.append(cl)

            def lstm_step(x_in):
                inp = x_in
                for l in range(L):
                    ps = pg.tile([H, 4], F32, tag="gates")
                    for k in range(4):
                        nc.tensor.matmul(ps[:, k:k + 1], wih[l][:, H * k:H * (k + 1)],
                                         inp[:], start=True, stop=False)
                        nc.tensor.matmul(ps[:, k:k + 1], whh[l][:, H * k:H * (k + 1)],
                                         h[l][:], start=False, stop=True)
                    sg = st.tile([H, 4], F32, tag="sg")
                    for k, fn in enumerate([AF.Sigmoid, AF.Sigmoid, AF.Tanh, AF.Sigmoid]):
                        nc.scalar.activation(sg[:, k:k + 1], ps[:, k:k + 1], fn,
                                             bias=bcol[:, 4 * l + k:4 * l + k + 1], scale=1.0)
                    t1 = st.tile([H, 1], F32, tag="t1")
                    nc.vector.tensor_mul(t1[:], sg[:, 1:2], c[l][:])
                    t2 = st.tile([H, 1], F32, tag="t2")
                    nc.vector.tensor_mul(t2[:], sg[:, 0:1], sg[:, 2:3])
                    cn = st.tile([H, 1], F32, tag=f"c{l}")
                    nc.vector.tensor_add(cn[:], t1[:], t2[:])
                    c[l] = cn
                    th = st.tile([H, 1], F32, tag="th")
                    nc.scalar.activation(th[:], cn[:], AF.Tanh, bias=0.0, scale=1.0)
                    hn = st.tile([H, 1], F32, tag=f"h{l}")
                    nc.vector.tensor_mul(hn[:], sg[:, 3:4], th[:])
                    h[l] = hn
                    inp = hn

            def sample(s_tile, oh_tile, g_tile, off, n, emb_tile):
                """Common tail: x_row = s + g; onehot = (x==max); embed select.

                s_tile[:, off:off+n] must already hold the tanh'd logits.
                Returns the new x column tile [H,1].
                """
                xr = st.tile([1, 16], F32, tag="xr")
                nc.vector.tensor_add(xr[:, :n], s_tile[:, off:off + n],
                                     g_tile[:, off:off + n])
                mx = st.tile([1, 1], F32, tag="mx")
                nc.vector.reduce_max(mx[:], xr[:, :n], axis=AX.X)
                nc.vector.tensor_scalar(oh_tile[:, off:off + n], xr[:, :n],
                                        scalar1=mx[:], scalar2=None,
                                        op0=ALU.is_equal)
                ohc_ps = pv.tile([16, 1], F32, tag="ohc")
                nc.tensor.matmul(ohc_ps[:n, :], oh_tile[:, off:off + n], ones[:],
                                 start=True, stop=True)
                ohc = st.tile([16, 1], F32, tag="ohc_sb")
                nc.scalar.copy(ohc[:n, :], ohc_ps[:n, :])
                xe = pv.tile([H, 1], F32, tag="v")
                nc.tensor.matmul(xe[:], emb_tile[:n, :], ohc[:n, :],
                                 start=True, stop=True)
                xn = st.tile([H, 1], F32, tag="x")
                nc.vector.tensor_copy(xn[:], xe[:])
                return xn

            for node in range(NUM_NODES):
                lstm_step(x)
                # attention key for this node: P[:, node] = W_ap @ h + b_ap
                pp = pv.tile([H, 1], F32, tag="v")
                nc.tensor.matmul(pp[:], wap[:], h[L - 1][:], start=True, stop=True)
                nc.scalar.add(P_sb[:, node:node + 1], pp[:], bap[:])
                if node > 0:
                    n = node
                    off = 16 * (node - 1)
                    aps = pv.tile([H, 1], F32, tag="v")
                    nc.tensor.matmul(aps[:], wac[:], h[L - 1][:], start=True, stop=True)
                    a2 = st.tile([H, 1], F32, tag="a2")
                    nc.scalar.add(a2[:], aps[:], bac[:])
                    qt = st.tile([H, 16], F32, tag="qt")
                    nc.vector.tensor_scalar_add(qt[:, :n], P_sb[:, :n], a2[:])
                    qth = st.tile([H, 16], F32, tag="qth")
                    nc.scalar.activation(qth[:, :n], qt[:, :n], AF.Tanh,
                                         bias=0.0, scale=1.0)
                    lg = pv.tile([1, 16], F32, tag="r")
                    nc.tensor.matmul(lg[:, :n], wpd[:], qth[:, :n],
                                     start=True, stop=True)
                    # t = tanh(raw/TEMP + b_pd/TEMP); s = TANH_C * t (host side)
                    nc.scalar.activation(tprev_sb[:, off:off + n], lg[:, :n],
                                         AF.Tanh, bias=bpd5[:], scale=1.0 / TEMP)
                    x = sample(tprev_sb, ohprev_sb, gprev, off, n, embp)
                lstm_step(x)
                off = NUM_OPS * node
                lo = pv.tile([1, 16], F32, tag="r")
                nc.tensor.matmul(lo[:, :NUM_OPS], h[L - 1][:], wod[:],
                                 start=True, stop=True)
                lob = st.tile([1, 16], F32, tag="lob")
                nc.vector.tensor_add(lob[:, :NUM_OPS], lo[:, :NUM_OPS], bodr[:])
                nc.scalar.activation(top_sb[:, off:off + NUM_OPS], lob[:, :NUM_OPS],
                                     AF.Tanh, bias=0.0, scale=1.0 / TEMP)
                x = sample(top_sb, ohop_sb, gop, off, NUM_OPS, embo)

            nc.sync.dma_start(out=o_tprev[:], in_=tprev_sb[:])
            nc.sync.dma_start(out=o_ohprev[:], in_=ohprev_sb[:])
            nc.sync.dma_start(out=o_top[:], in_=top_sb[:])
            nc.sync.dma_start(out=o_ohop[:], in_=ohop_sb[:])

    nc.compile()
    return nc


def _extract_gumbels(sample_key):
    """Replicate the reference's key-split order; return gumbels scaled 1/TANH_C."""
    import jax
    import jax.numpy as jnp

    cpu = jax.devices("cpu")[0]
    with jax.default_device(cpu):
        key = jnp.asarray(np.asarray(sample_key).view(np.uint32))
        g_prev = np.zeros((NUM_NODES - 1, 16), np.float32)
        g_op = np.zeros((NUM_NODES, NUM_OPS), np.float32)
        for node_id in range(NUM_NODES):
            if node_id > 0:
                key, sub = jax.random.split(key)
                g = np.asarray(jax.random.gumbel(sub, (node_id,), jnp.float32))
                g_prev[node_id - 1, :node_id] = g / TANH_C
            key, sub = jax.random.split(key)
            g = np.asarray(jax.random.gumbel(sub, (NUM_OPS,), jnp.float32))
            g_op[node_id] = g / TANH_C
    return g_prev, g_op


def make_in_map(W_ih, W_hh, b_ih, b_hh, W_ap, b_ap, W_ac, b_ac, W_pd, b_pd,
                W_od, b_od, emb_prev, emb_op, sample_key):
    """Host-side input prep: transposes, bias packing, gumbel extraction."""
    f = np.float32
    W_ih, W_hh = np.asarray(W_ih, f), np.asarray(W_hh, f)
    b = (np.asarray(b_ih, f) + np.asarray(b_hh, f))  # [L, 4H]
    g_prev, g_op = _extract_gumbels(sample_key)
    return {
        "wih_t": np.ascontiguousarray(W_ih.transpose(0, 2, 1)),
        "whh_t": np.ascontiguousarray(W_hh.transpose(0, 2, 1)),
        # b_col[:, 4l+k] = b[l, H*k:H*(k+1)]
        "b_col": np.ascontiguousarray(
            b.reshape(L, 4, H).transpose(2, 0, 1).reshape(H, 4 * L)),
        "wap_t": np.ascontiguousarray(np.asarray(W_ap, f).T),
        "bap_col": np.asarray(b_ap, f).reshape(H, 1),
        "wac_t": np.ascontiguousarray(np.asarray(W_ac, f).T),
        "bac_col": np.asarray(b_ac, f).reshape(H, 1),
        "wpd_col": np.ascontiguousarray(np.asarray(W_pd, f).reshape(1, H).T),
        "bpd5": (np.asarray(b_pd, f) / TEMP).reshape(1, 1),
        "wod_t": np.ascontiguousarray(np.asarray(W_od, f).T),
        "bod_row": np.asarray(b_od, f).reshape(1, NUM_OPS),
        "embp": np.asarray(emb_prev, f),
        "embo": np.asarray(emb_op, f),
        "gprev": g_prev.reshape(1, -1),
        "gop": g_op.reshape(1, -1),
    }


def postprocess(out_map):
    """argmax indices + log-softmax log_prob from the device's s/onehot rows."""
    t_prev = out_map["t_prev"].reshape(NUM_NODES - 1, 16)
    oh_prev = out_map["oh_prev"].reshape(NUM_NODES - 1, 16)
    t_op = out_map["t_op"].reshape(NUM_NODES, NUM_OPS)
    oh_op = out_map["oh_op"].reshape(NUM_NODES, NUM_OPS)

    lp = 0.0
    prev_ids = np.zeros(NUM_NODES - 1, np.int32)
    ops = np.zeros(NUM_NODES, np.int32)

    def logsoftmax_at(t_row, idx):
        s = TANH_C * t_row.astype(np.float64)
        m = s.max()
        return s[idx] - (m + np.log(np.exp(s - m).sum()))

    for e in range(NUM_NODES - 1):
        n = e + 1
        pid = int(np.argmax(oh_prev[e, :n]))
        prev_ids[e] = pid
        lp += logsoftmax_at(t_prev[e, :n], pid)
    for e in range(NUM_NODES):
        op = int(np.argmax(oh_op[e]))
        ops[e] = op
        lp += logsoftmax_at(t_op[e], op)
    return prev_ids, ops, np.float32(lp)


def kernel(**inputs):
    if "nc" not in _CACHE:
        _CACHE["nc"] = _build_nc()
    nc = _CACHE["nc"]
    in_map = make_in_map(**inputs)
    res = run_bass_kernel_spmd(nc, [in_map] * N_CORES, list(range(N_CORES)))
    return postprocess(res.results[0])
